# revision 1
# baseline (speedup 1.0000x reference)
"""AnchorGenerator on 8 TRN2 NeuronCores.

The reference output depends only on H=W=512 (feature_map values are unused):
for each (y, x, s, r) the anchor row is
    [max(16x+8-hw, 0), max(16y+8-hh, 0), min(16x+8+hw, 8192), min(16y+8+hh, 8192)]
with hw/hh the 3x3 half-width/height tables.

Sharding: 64 grid rows per core. Per core the flat (294912, 4) f32 output slab
is exactly a [128, 9216] SBUF tile in partition-major order, with
partition p = (y_rel, x_half) and free index f = x_rel*36 + (s*3+r)*4 + c.

The unclamped value decomposes as a low-rank product
    v[p, f] = C[f] + Ygrid[p]*my[f] + X[p]*mx[f]
with the per-core row offset folded into C.  Clamping only bites at grid
edges (x=0 / x=511 columns everywhere; y=0 rows on core 0; y=511 rows on
core 7) and each case is itself rank-1: (partition indicator) x (column
correction).  Folding those four corrections in as extra contraction rows
gives a K=8 bf16 matmul whose fp32 PSUM output IS the clamped result
(PE cost scales with N, not K), so no elementwise clamp is needed at all.

Per chunk the PSUM result is plain-copied to SBUF alternating between DVE
(tensor_copy) and ACT (activation Copy) so the copy never gates the DMA
stream, and contiguous HWDGE stores alternate over both rings (SP for odd
chunks, ACT for even).  Tables arrive via two early HWDGE loads on the
SP/ACT engines (idle at kernel start, ~0.6us first-byte) instead of SWDGE
on gpsimd (which is busy with framework memsets).  Per-core HBM traffic is
the 4.72 MB output plus ~150 KB of tables.

hw/hh are computed with jnp on the device (mirroring the reference's op
sequence) so non-IEEE sqrt/divide rounding matches the reference bit-for-bit.
"""

import numpy as np
import ml_dtypes

H = 512
W = 512
N_CORES = 8
ROWS_PER_CORE = H // N_CORES  # 64
P = 128                       # partitions = (y_rel, x_half)
XW = W // 2                   # 256 x-positions per partition
SR = 9                        # scale x ratio combos
FREE = XW * SR * 4            # 9216 floats per partition
K = 8                         # bf16 contraction: C1,C2,Ygrid,X + 4 edge fixups
CHUNKS = (256, 512, 1024, 1024, 1024, 1024, 1024, 1024, 1024, 1024, 256)
HEAD_CHUNKS = 4               # chunks covered by the first (fast SWDGE) load
# stores are grouped into fewer, wider transfers (longer DRAM lines amortize
# per-packet overhead on the straggling SDMA engine 0)
STORE_GROUPS = ((0,), (1,), (2,), (3,), (4, 5), (6, 7), (8, 9, 10))
MM_N = 512                    # matmul free-dim (one PSUM bank)
PACKED = P + FREE             # lhsT columns then rhs columns, one input

_cache = {}


def _bf16_split(v, n):
    """Split f64 vector v into n bf16 addends, most-significant first."""
    parts = []
    rem = v.copy()
    for _ in range(n):
        p = rem.astype(ml_dtypes.bfloat16)
        parts.append(p)
        rem = rem - p.astype(np.float64)
    return parts


def _half_sizes():
    """hw, hh as (3,3) f32, matching the reference's jnp ops on this backend."""
    import jax.numpy as jnp

    scales = jnp.asarray((0.5, 1.0, 2.0), dtype=jnp.float32)
    ratios = jnp.asarray((0.5, 1.0, 2.0), dtype=jnp.float32)
    sqrt_r = jnp.sqrt(ratios)
    aw = 16.0 * scales[:, None] * sqrt_r[None, :]
    ah = 16.0 * scales[:, None] / sqrt_r[None, :]
    hw = np.asarray(aw / 2, dtype=np.float32)
    hh = np.asarray(ah / 2, dtype=np.float32)
    return hw, hh


def _tables():
    """Per-core packed bf16 input (K, PACKED): lhsT columns then rhs columns."""
    hw, hh = _half_sizes()
    off = np.stack([-hw, -hh, hw, hh], axis=-1).reshape(36).astype(np.float64)
    isx = np.tile(np.array([1.0, 0.0, 1.0, 0.0]), SR)  # c parity: x-coords even
    x_rel = np.arange(XW, dtype=np.float64)
    base = 8.0 + 16.0 * x_rel[:, None] * isx[None, :]  # (XW, 36)
    mx = np.broadcast_to(isx, (XW, 36)).reshape(FREE)
    my = 1.0 - mx
    my_b = my.astype(ml_dtypes.bfloat16)
    mx_b = mx.astype(ml_dtypes.bfloat16)

    p = np.arange(P)
    Ygrid = (16.0 * (p // 2)).astype(ml_dtypes.bfloat16)   # exact
    X = (4096.0 * (p % 2)).astype(ml_dtypes.bfloat16)      # exact
    ones = np.ones(P, ml_dtypes.bfloat16)
    Iy0 = (p < 2).astype(ml_dtypes.bfloat16)               # y == 0 rows
    Iyt = (p >= P - 2).astype(ml_dtypes.bfloat16)          # y == 511 rows
    Iev = (1 - p % 2).astype(ml_dtypes.bfloat16)           # x_half == 0
    Iod = (p % 2).astype(ml_dtypes.bfloat16)               # x_half == 1

    f = np.arange(FREE)
    cpos = f % 4
    hh_f = hh.reshape(9)[(f // 4) % 9].astype(np.float64)
    hw_f = hw.reshape(9)[(f // 4) % 9].astype(np.float64)

    packed = np.zeros((N_CORES, K, PACKED), ml_dtypes.bfloat16)
    for c in range(N_CORES):
        # fold the per-core row offset into C's y-columns
        Cc = (base + off[None, :] + 1024.0 * c * (1.0 - isx)[None, :]).reshape(FREE)
        C1, C2 = _bf16_split(Cc, 2)
        Cb = C1.astype(np.float64) + C2.astype(np.float64)
        # edge fixups: exact clamped value minus the unclamped rank-3 sum
        ey0 = np.where((cpos == 1) & (c == 0) & (8 - hh_f < 0), -Cb, 0.0)
        eyt = np.where(
            (cpos == 3) & (c == N_CORES - 1) & (8184 + hh_f > 8192),
            8192.0 - (Cb + 1008.0), 0.0,
        )
        exl = np.where((cpos == 0) & (f < 36) & (8 - hw_f < 0), -Cb, 0.0)
        exr = np.where(
            (cpos == 2) & (f >= FREE - 36) & (8184 + hw_f > 8192),
            8192.0 - (Cb + 4096.0), 0.0,
        )
        packed[c, :, :P] = np.stack([ones, ones, Ygrid, X, Iy0, Iyt, Iev, Iod])
        packed[c, :, P:] = np.stack([
            C1, C2, my_b, mx_b,
            ey0.astype(ml_dtypes.bfloat16), eyt.astype(ml_dtypes.bfloat16),
            exl.astype(ml_dtypes.bfloat16), exr.astype(ml_dtypes.bfloat16),
        ])
    return packed


def build_nc():
    import contextlib

    import concourse.bacc as bacc
    import concourse.mybir as mybir
    import concourse.tile as tile

    nc = bacc.Bacc(None)
    tabs_d = nc.declare_dram_parameter("tabs", [K, PACKED], mybir.dt.bfloat16, isOutput=False)
    out_d = nc.declare_dram_parameter("out", [P, FREE], mybir.dt.float32, isOutput=True)

    headc = sum(CHUNKS[:HEAD_CHUNKS])
    headn = P + headc
    # The head table (lhsT + chunks 0-3) loads via SWDGE in the MAIN block,
    # before the tile context: its transfer overlaps the framework preamble
    # and tile-entry barrier.  A PE wait_ge in main (outside the scheduled
    # tile block, so the tile scheduler's sim never sees it) holds the whole
    # tile entry until the head has landed -- by which time the matmuls can
    # start immediately.  The rest loads as a normal tile-tracked DMA.
    es = contextlib.ExitStack()
    headbuf = es.enter_context(
        nc.sbuf_tensor("tabs_head", [K, headn], mybir.dt.bfloat16)
    )
    _cache.setdefault("es", []).append(es)  # keep the allocation alive
    s_head = nc.alloc_semaphore("tab_head")
    nc.gpsimd.dma_start(headbuf[:, :], tabs_d[:, :headn]).then_inc(s_head, 16)
    nc.tensor.wait_ge(s_head, 16)

    chunk_off = [sum(CHUNKS[:i]) for i in range(len(CHUNKS) + 1)]
    with tile.TileContext(nc) as tc:
        with (
            tc.tile_pool(name="const", bufs=1) as cpool,
            tc.tile_pool(name="osb", bufs=1) as osb,
            tc.tile_pool(name="psb", bufs=3, space="PSUM") as psb,
            tc.tile_pool(name="pss", bufs=2, space="PSUM") as pss,
        ):
            rest = cpool.tile([K, FREE - headc], mybir.dt.bfloat16)
            nc.gpsimd.dma_start(rest[:, :], tabs_d[:, headn:])
            lhsT = headbuf[:, :P]
            obuf = osb.tile([P, FREE], mybir.dt.float32)
            for ci, size in enumerate(CHUNKS):
                lo = chunk_off[ci]
                pool = pss if size < 1024 else psb
                acc = pool.tile([P, size], mybir.dt.float32,
                                tag="accs" if size < 1024 else "accb")
                for m0 in range(0, size, MM_N):
                    n = min(MM_N, size - m0)
                    if ci < HEAD_CHUNKS:
                        rhs = headbuf[:, P + lo + m0: P + lo + m0 + n]
                    else:
                        rhs = rest[:, lo - headc + m0: lo - headc + m0 + n]
                    nc.tensor.matmul(acc[:, m0:m0 + n], lhsT, rhs)
                o = obuf[:, lo:lo + size]
                if ci % 2 == 0:
                    nc.vector.tensor_copy(o, acc[:])
                else:
                    nc.scalar.copy(o, acc[:])
                # emit the store whose group this chunk completes; all stores
                # on the otherwise-idle SP sequencer
                for grp in STORE_GROUPS:
                    if grp[-1] == ci:
                        glo, ghi = chunk_off[grp[0]], chunk_off[ci + 1]
                        nc.sync.dma_start(out_d[:, glo:ghi], obuf[:, glo:ghi])
    nc.compile()
    return nc


def kernel(feature_map: np.ndarray) -> np.ndarray:
    from concourse.bass_utils import run_bass_kernel_spmd

    if "tables" not in _cache:
        _cache["tables"] = _tables()
    packed = _cache["tables"]
    if "nc" not in _cache:
        _cache["nc"] = build_nc()
    nc = _cache["nc"]

    in_maps = [{"tabs": packed[c]} for c in range(N_CORES)]
    res = run_bass_kernel_spmd(nc, in_maps, core_ids=list(range(N_CORES)))
    return np.concatenate(
        [res.results[c]["out"].reshape(-1, 4) for c in range(N_CORES)], axis=0
    )



# revision 2
# speedup vs baseline: 1.2781x; 1.2781x over previous
"""AnchorGenerator on 8 TRN2 NeuronCores.

The reference output depends only on H=W=512 (feature_map values are unused):
for each (y, x, s, r) the anchor row is
    [max(16x+8-hw, 0), max(16y+8-hh, 0), min(16x+8+hw, 8192), min(16y+8+hh, 8192)]
with hw/hh the 3x3 half-width/height tables.

Sharding: 64 grid rows per core. Per core the flat (294912, 4) output slab
is exactly a [128, 9216] SBUF tile in partition-major order, with
partition p = (y_rel, x_half) and free index f = x_rel*36 + (s*3+r)*4 + c.

The unclamped value decomposes as a low-rank product
    v[p, f] = C[f] + Ygrid[p]*my[f] + X[p]*mx[f]
with the per-core row offset folded into C.  Clamping only bites at grid
edges and each case is itself rank-1, so a K=8 bf16 matmul's fp32 PSUM
output IS the clamped result (PE cost scales with N, not K).

Output is stored as bf16 and upcast to f32 on the host: anchor values are
<= 8192 so bf16 rounding is <= 16 absolute (= 2e-3 of scale, 0.4%
per-element), far inside tolerance, and it halves HBM store traffic
(2.36 MB/core -> ~6.6 us at 358 GB/s).

Startup: the [8, PACKED] bf16 table spans partitions 0-7, which map to only
2 of the 16 SDMA engines, so one monolithic load is slow; SWDGE (gpsimd)
additionally pays ~1 us first-byte and queues behind framework memsets.
Instead the head (lhsT + first 3 chunks) loads via HWDGE on the idle SP
sequencer in the MAIN block (before tile entry; a PE wait_ge holds the
matmuls), and the remaining table streams in 3 more tile-tracked HWDGE
loads that stay ahead of PE consumption (~19 GB/s needed vs ~50 available).

Per chunk the PSUM result is cast-copied (f32 -> bf16) to SBUF alternating
DVE / ACT so the copy never gates the DMA stream; contiguous HWDGE stores
on SP drain 7 groups sized >= 512 B/partition-descriptor.
"""

import numpy as np
import ml_dtypes

H = 512
W = 512
N_CORES = 8
ROWS_PER_CORE = H // N_CORES  # 64
P = 128                       # partitions = (y_rel, x_half)
XW = W // 2                   # 256 x-positions per partition
SR = 9                        # scale x ratio combos
FREE = XW * SR * 4            # 9216 values per partition
K = 8                         # bf16 contraction: C1,C2,Ygrid,X + 4 edge fixups
PACKED = P + FREE             # lhsT columns then rhs columns, one input
MM_N = 512                    # matmul free-dim (one PSUM bank)

CHUNKS = (256, 256, 512, 1024, 1024, 1024, 1024, 1024, 1024, 1024,
          512, 256, 256)      # sums to FREE
HEAD_CHUNKS = 3               # chunks covered by the main-block head load
# tile-tracked loads of the rest of the table, as (chunk_lo, chunk_hi)
LOAD_GROUPS = ((3, 5), (5, 8), (8, 13))
# stores grouped so every descriptor is >= 512 B/partition in bf16
STORE_GROUPS = ((0,), (1, 2), (3,), (4, 5), (6, 7), (8, 9), (10, 11, 12))

_cache = {}


def _bf16_split(v, n):
    """Split f64 vector v into n bf16 addends, most-significant first."""
    parts = []
    rem = v.copy()
    for _ in range(n):
        p = rem.astype(ml_dtypes.bfloat16)
        parts.append(p)
        rem = rem - p.astype(np.float64)
    return parts


def _half_sizes():
    """hw, hh as (3,3) f32, matching the reference's jnp ops on this backend."""
    import jax.numpy as jnp

    scales = jnp.asarray((0.5, 1.0, 2.0), dtype=jnp.float32)
    ratios = jnp.asarray((0.5, 1.0, 2.0), dtype=jnp.float32)
    sqrt_r = jnp.sqrt(ratios)
    aw = 16.0 * scales[:, None] * sqrt_r[None, :]
    ah = 16.0 * scales[:, None] / sqrt_r[None, :]
    hw = np.asarray(aw / 2, dtype=np.float32)
    hh = np.asarray(ah / 2, dtype=np.float32)
    return hw, hh


def _tables():
    """Per-core packed bf16 input (K, PACKED): lhsT columns then rhs columns."""
    hw, hh = _half_sizes()
    off = np.stack([-hw, -hh, hw, hh], axis=-1).reshape(36).astype(np.float64)
    isx = np.tile(np.array([1.0, 0.0, 1.0, 0.0]), SR)  # c parity: x-coords even
    x_rel = np.arange(XW, dtype=np.float64)
    base = 8.0 + 16.0 * x_rel[:, None] * isx[None, :]  # (XW, 36)
    mx = np.broadcast_to(isx, (XW, 36)).reshape(FREE)
    my = 1.0 - mx
    my_b = my.astype(ml_dtypes.bfloat16)
    mx_b = mx.astype(ml_dtypes.bfloat16)

    p = np.arange(P)
    Ygrid = (16.0 * (p // 2)).astype(ml_dtypes.bfloat16)   # exact
    X = (4096.0 * (p % 2)).astype(ml_dtypes.bfloat16)      # exact
    ones = np.ones(P, ml_dtypes.bfloat16)
    Iy0 = (p < 2).astype(ml_dtypes.bfloat16)               # y == 0 rows
    Iyt = (p >= P - 2).astype(ml_dtypes.bfloat16)          # y == 511 rows
    Iev = (1 - p % 2).astype(ml_dtypes.bfloat16)           # x_half == 0
    Iod = (p % 2).astype(ml_dtypes.bfloat16)               # x_half == 1

    f = np.arange(FREE)
    cpos = f % 4
    hh_f = hh.reshape(9)[(f // 4) % 9].astype(np.float64)
    hw_f = hw.reshape(9)[(f // 4) % 9].astype(np.float64)

    packed = np.zeros((N_CORES, K, PACKED), ml_dtypes.bfloat16)
    for c in range(N_CORES):
        # fold the per-core row offset into C's y-columns
        Cc = (base + off[None, :] + 1024.0 * c * (1.0 - isx)[None, :]).reshape(FREE)
        C1, C2 = _bf16_split(Cc, 2)
        Cb = C1.astype(np.float64) + C2.astype(np.float64)
        # edge fixups: exact clamped value minus the unclamped rank-3 sum
        ey0 = np.where((cpos == 1) & (c == 0) & (8 - hh_f < 0), -Cb, 0.0)
        eyt = np.where(
            (cpos == 3) & (c == N_CORES - 1) & (8184 + hh_f > 8192),
            8192.0 - (Cb + 1008.0), 0.0,
        )
        exl = np.where((cpos == 0) & (f < 36) & (8 - hw_f < 0), -Cb, 0.0)
        exr = np.where(
            (cpos == 2) & (f >= FREE - 36) & (8184 + hw_f > 8192),
            8192.0 - (Cb + 4096.0), 0.0,
        )
        packed[c, :, :P] = np.stack([ones, ones, Ygrid, X, Iy0, Iyt, Iev, Iod])
        packed[c, :, P:] = np.stack([
            C1, C2, my_b, mx_b,
            ey0.astype(ml_dtypes.bfloat16), eyt.astype(ml_dtypes.bfloat16),
            exl.astype(ml_dtypes.bfloat16), exr.astype(ml_dtypes.bfloat16),
        ])
    return packed


def build_nc():
    import contextlib

    import concourse.bacc as bacc
    import concourse.mybir as mybir
    import concourse.tile as tile

    nc = bacc.Bacc(None)
    tabs_d = nc.declare_dram_parameter("tabs", [K, PACKED], mybir.dt.bfloat16, isOutput=False)
    out_d = nc.declare_dram_parameter("out", [P, FREE], mybir.dt.bfloat16, isOutput=True)

    chunk_off = [sum(CHUNKS[:i]) for i in range(len(CHUNKS) + 1)]
    headc = chunk_off[HEAD_CHUNKS]   # rhs cols in the head load
    headn = P + headc

    # Head load (lhsT + chunks 0-2) via HWDGE on the idle SP sequencer in the
    # MAIN block: its ~0.6us first-byte overlaps the framework preamble and
    # tile-entry barrier.  A PE wait_ge in main (outside the scheduled tile
    # block) holds the matmuls until the head has landed.
    es = contextlib.ExitStack()
    headbuf = es.enter_context(
        nc.sbuf_tensor("tabs_head", [K, headn], mybir.dt.bfloat16)
    )
    _cache.setdefault("es", []).append(es)  # keep the allocation alive
    s_head = nc.alloc_semaphore("tab_head")
    nc.sync.dma_start(headbuf[:, :], tabs_d[:, :headn]).then_inc(s_head, 16)
    nc.tensor.wait_ge(s_head, 16)

    with tile.TileContext(nc) as tc:
        with (
            tc.tile_pool(name="const", bufs=1) as cpool,
            tc.tile_pool(name="osb", bufs=1) as osb,
            tc.tile_pool(name="psb", bufs=3, space="PSUM") as psb,
            tc.tile_pool(name="pss", bufs=2, space="PSUM") as pss,
        ):
            rest = cpool.tile([K, FREE - headc], mybir.dt.bfloat16)
            for lo_c, hi_c in LOAD_GROUPS:
                a = chunk_off[lo_c] - headc
                b = chunk_off[hi_c] - headc
                nc.sync.dma_start(rest[:, a:b], tabs_d[:, headn + a:headn + b])
            lhsT = headbuf[:, :P]
            obuf = osb.tile([P, FREE], mybir.dt.bfloat16)
            for ci, size in enumerate(CHUNKS):
                lo = chunk_off[ci]
                pool = pss if size <= MM_N else psb
                acc = pool.tile([P, size], mybir.dt.float32,
                                tag="accs" if size <= MM_N else "accb")
                for m0 in range(0, size, MM_N):
                    n = min(MM_N, size - m0)
                    if ci < HEAD_CHUNKS:
                        rhs = headbuf[:, P + lo + m0: P + lo + m0 + n]
                    else:
                        rhs = rest[:, lo - headc + m0: lo - headc + m0 + n]
                    nc.tensor.matmul(acc[:, m0:m0 + n], lhsT, rhs)
                o = obuf[:, lo:lo + size]
                if ci % 2 == 0:
                    nc.vector.tensor_copy(o, acc[:])
                else:
                    nc.scalar.copy(o, acc[:])
                # emit the store whose group this chunk completes
                for grp in STORE_GROUPS:
                    if grp[-1] == ci:
                        glo, ghi = chunk_off[grp[0]], chunk_off[ci + 1]
                        nc.sync.dma_start(out_d[:, glo:ghi], obuf[:, glo:ghi])
    nc.compile()
    return nc


def kernel(feature_map: np.ndarray) -> np.ndarray:
    from concourse.bass_utils import run_bass_kernel_spmd

    if "tables" not in _cache:
        _cache["tables"] = _tables()
    packed = _cache["tables"]
    if "nc" not in _cache:
        _cache["nc"] = build_nc()
    nc = _cache["nc"]

    in_maps = [{"tabs": packed[c]} for c in range(N_CORES)]
    res = run_bass_kernel_spmd(nc, in_maps, core_ids=list(range(N_CORES)))
    return np.concatenate(
        [np.asarray(res.results[c]["out"], dtype=np.float32).reshape(-1, 4)
         for c in range(N_CORES)],
        axis=0,
    )
